# revision 1
# baseline (speedup 1.0000x reference)
"""CopyGenerator kernel for Trainium2 (Bass/Tile), vocab-parallel over 8 cores.

Per core c (vocab shard [c*4000, (c+1)*4000), attention batch c):
  attention for OWN batch only -> attnT_own, a_own; AllGather (33KB) shares
  all batches' attnT/a with every core (latency hidden under pass 1).
  gen_score = htgt @ emb_shard.T                       (PE, fp16 in / fp32 acc)
  e = exp(gen_score)   [no max-sub; scores are O(3)]   (ACT, fused row-sum)
  Z = allreduce_add(sum_v e), split into two batch groups so pass 2 of
      group 0 overlaps pass 1 of group 1.
  copy_p shard = attn @ onehot(src_local)              (PE, fp16 exact onehot)
  out = log(a*copy_p + (1-a)*e/Z) = Ln(c1*(c2*copy_p + e)),
      c1=(1-a)/Z, c2=a*Z/(1-a)

All transposed operands (embT, htgtT, hh_own, qwT) are produced on-chip via
PE transpose from natural-layout DMA loads (4-byte-stride DMA loads are ~40x
slower than row-major), cast fp32->fp16 on the PSUM->SBUF copy.
"""

import os
import sys

sys.path.insert(0, "/opt/trn_rl_repo")

import numpy as np

from concourse import bass, bacc, mybir
import concourse.tile as tile
from concourse.bass_utils import run_bass_kernel_spmd
from concourse.masks import make_identity

NT, NS, BS, D, V = 128, 128, 8, 512, 32000
NCORES = 8
VS = V // NCORES  # 4000 vocab per core
NCH = 8
CH = VS // NCH  # 500 cols per chunk (one PSUM bank)
VT = 4  # v-subtiles per chunk for emb transpose
CVT = CH // VT  # 125 rows per emb transpose block
P = 128
KC = D // P  # 4 contraction chunks
NG = 2  # Z-collective batch groups
GB = BS // NG  # batches per group
F32 = mybir.dt.float32
F16 = mybir.dt.float16
I16 = mybir.dt.int16
AF = mybir.ActivationFunctionType
ALU = mybir.AluOpType
INV_SQRT_D = 1.0 / float(np.sqrt(np.float32(D)))
AGW = NT + 2  # allgather row width: attnT row (t) + a (1 fp32 = 2 fp16)


def build_kernel():
    nc = bacc.Bacc(
        "TRN2",
        target_bir_lowering=False,
        debug=False,
        enable_asserts=False,
        num_devices=NCORES,
    )
    htgt = nc.dram_tensor("htgt", [NT, BS, D], F32, kind="ExternalInput").ap()
    htgt_own = nc.dram_tensor("htgt_own", [NT, D], F32, kind="ExternalInput").ap()
    hsrc_own = nc.dram_tensor("hsrc_own", [NS, D], F32, kind="ExternalInput").ap()
    src = nc.dram_tensor("src_local", [NS, BS], F32, kind="ExternalInput").ap()
    emb = nc.dram_tensor("emb", [VS, D], F32, kind="ExternalInput").ap()
    q_w = nc.dram_tensor("q_w", [D, D], F32, kind="ExternalInput").ap()
    q_b = nc.dram_tensor("q_b", [D], F32, kind="ExternalInput").ap()
    f_w = nc.dram_tensor("f_w", [D, D], F32, kind="ExternalInput").ap()
    f_b = nc.dram_tensor("f_b", [D], F32, kind="ExternalInput").ap()
    copy_w = nc.dram_tensor("copy_w", [1, D], F32, kind="ExternalInput").ap()
    copy_b = nc.dram_tensor("copy_b", [1], F32, kind="ExternalInput").ap()
    out = nc.dram_tensor("out", [NT, BS, VS], F32, kind="ExternalOutput").ap()

    with tile.TileContext(nc) as tc:
        _emit(
            nc, tc, htgt, htgt_own, hsrc_own, src, emb, q_w, q_b, f_w, f_b,
            copy_w, copy_b, out,
        )
    nc.compile()
    return nc


def _emit(
    nc, tc, htgt, htgt_own, hsrc_own, src, emb, q_w, q_b, f_w, f_b,
    copy_w, copy_b, out,
):
    ablate = os.environ.get("KABLATE", "full")
    with (
        tc.tile_pool(name="persist", bufs=1) as pw,
        tc.tile_pool(name="small", bufs=2) as psm,
        tc.tile_pool(name="ps_attn", bufs=2, space="PSUM") as ps_at,
        tc.tile_pool(name="ps_tr", bufs=2, space="PSUM") as ps_tr,
        tc.tile_pool(name="ps_gen", bufs=4, space="PSUM") as ps_gen,
        tc.tile_pool(name="dram", bufs=1, space="DRAM") as pdram,
    ):
        # ---- persistent SBUF ----
        htgtT = pw.tile([P, KC, BS, P], F16)  # (d, kc, b, t)
        hh_own = pw.tile([P, KC, 2, P], F16)  # (d, kc, {tgt,src}, t/s)
        qwT = pw.tile([P, KC, D], F16)  # (d, kc, i)
        embT = pw.tile([P, KC, VS], F16)  # (d, kc, v)
        attnT_all = pw.tile([P, BS, NT], F16)  # (s, b, t)
        a_all = pw.tile([P, BS], F32)
        src_sb = pw.tile([P, BS], F32)
        iota_all = pw.tile([P, NCH, CH], I16)
        w2_sb = pw.tile([P, KC], F32)
        b2_sb = pw.tile([1, 1], F32)
        identity = pw.tile([P, P], F32)
        ones16 = pw.tile([1, 2 * P], F16)
        ones32 = pw.tile([1, P], F32)
        qb_row = pw.tile([1, D], F32)
        qb16 = pw.tile([1, D], F16)
        zparts = pw.tile([P, BS, NCH], F32)
        zloc = pw.tile([P, BS], F32)
        zg_sb = pw.tile([P, BS], F32)
        ag_pack = pw.tile([P, AGW], F16)  # (s, t | a-bits)

        ag_in = pdram.tile([P, AGW], F16)
        ag_out = pdram.tile([NCORES * P, AGW], F16)
        zin = [pdram.tile([P, GB], F32, name=f"zin{g}") for g in range(NG)]
        zout = [pdram.tile([P, GB], F32, name=f"zout{g}") for g in range(NG)]

        make_identity(nc, identity[:])
        nc.vector.memset(ones16[:], 1.0)
        nc.vector.memset(ones32[:], 1.0)
        nc.sync.dma_start(out=src_sb[:], in_=src)
        for n in range(NCH):
            nc.gpsimd.iota(
                iota_all[:, n, :],
                pattern=[[1, CH]],
                base=n * CH,
                channel_multiplier=0,
            )

        # ---- embT: load+transpose early so gen can start asap ----
        with tc.tile_pool(name="embn", bufs=2) as pembn:
            emb_r = emb.rearrange("(n vt v) d -> v n vt d", v=CVT, vt=VT)
            for n in range(NCH):
                e_nat = pembn.tile([CVT, VT, D], F32, tag="enat")
                nc.sync.dma_start(out=e_nat[:], in_=emb_r[:, n])
                for vt in range(VT):
                    v0 = n * CH + vt * CVT
                    t_ps = ps_tr.tile([P, KC * CVT], F32, tag="tr")
                    for kc in range(KC):
                        nc.tensor.transpose(
                            t_ps[:, kc * CVT : (kc + 1) * CVT],
                            e_nat[:, vt, kc * P : (kc + 1) * P],
                            identity[0:CVT, 0:CVT],
                        )
                    cp = nc.scalar.copy if vt % 2 == 0 else nc.vector.tensor_copy
                    cp(
                        out=embT[:, :, v0 : v0 + CVT],
                        in_=t_ps[:].rearrange("d (kc v) -> d kc v", v=CVT),
                    )

        # ---- loads: natural DMA + PE transpose (into one PSUM bank) + fp16 cast
        with tc.tile_pool(name="nat", bufs=4) as pnat:
            for b in range(BS):
                h_nat = pnat.tile([P, D], F32, tag="hnat")
                nc.sync.dma_start(out=h_nat[:], in_=htgt[:, b, :])
                t_ps = ps_tr.tile([P, D], F32, tag="tr")
                for kc in range(KC):
                    nc.tensor.transpose(
                        t_ps[:, kc * P : (kc + 1) * P],
                        h_nat[:, kc * P : (kc + 1) * P],
                        identity[:],
                    )
                nc.vector.tensor_copy(
                    out=htgtT[:, :, b, :],
                    in_=t_ps[:].rearrange("d (kc t) -> d kc t", t=P),
                )
            for which, t_dram in ((0, htgt_own), (1, hsrc_own)):
                h_nat = pnat.tile([P, D], F32, tag="hnat")
                nc.sync.dma_start(out=h_nat[:], in_=t_dram)
                t_ps = ps_tr.tile([P, D], F32, tag="tr")
                for kc in range(KC):
                    nc.tensor.transpose(
                        t_ps[:, kc * P : (kc + 1) * P],
                        h_nat[:, kc * P : (kc + 1) * P],
                        identity[:],
                    )
                nc.vector.tensor_copy(
                    out=hh_own[:, :, which, :],
                    in_=t_ps[:].rearrange("d (kc t) -> d kc t", t=P),
                )
            for ic in range(KC):
                qw_nat = pnat.tile([P, D], F32, tag="qwnat")
                nc.sync.dma_start(out=qw_nat[:], in_=q_w[ic * P : (ic + 1) * P, :])
                t_ps = ps_tr.tile([P, D], F32, tag="tr")
                for kc in range(KC):
                    nc.tensor.transpose(
                        t_ps[:, kc * P : (kc + 1) * P],
                        qw_nat[:, kc * P : (kc + 1) * P],
                        identity[:],
                    )
                nc.vector.tensor_copy(
                    out=qwT[:, :, ic * P : (ic + 1) * P],
                    in_=t_ps[:].rearrange("d (kc i) -> d kc i", i=P),
                )

        nc.sync.dma_start(out=qb_row[:], in_=q_b.unsqueeze(0))
        nc.vector.tensor_copy(out=qb16[:], in_=qb_row[:])

        # ---- fold f_w/copy_w: w2 = f_w.T @ copy_w.T ; b2 = copy_w@f_b + copy_b
        with tc.tile_pool(name="fwp", bufs=1) as pfw:
            fw = pfw.tile([P, KC, D], F32)  # (j, jc, i)
            cwT = pfw.tile([P, KC], F32)  # (j, jc)
            fbT = pfw.tile([P, KC], F32)  # (j, jc)
            cb_sb = pfw.tile([1, 1], F32)
            fw_r = f_w.rearrange("(jc j) i -> j jc i", j=P)
            for kc in range(KC):
                nc.sync.dma_start(out=fw[:, kc], in_=fw_r[:, kc])
            nc.sync.dma_start(
                out=cwT[:], in_=copy_w.rearrange("o (jc j) -> j (jc o)", j=P)
            )
            nc.sync.dma_start(out=fbT[:], in_=f_b.rearrange("(jc j) -> j jc", j=P))
            nc.sync.dma_start(out=cb_sb[:], in_=copy_b.unsqueeze(0))
            for ic in range(KC):
                w2_ps = ps_at.tile([P, 1], F32, tag="at")
                for jc in range(KC):
                    nc.tensor.matmul(
                        out=w2_ps[:],
                        lhsT=fw[:, jc, ic * P : (ic + 1) * P],
                        rhs=cwT[:, jc : jc + 1],
                        start=(jc == 0),
                        stop=(jc == KC - 1),
                    )
                nc.vector.tensor_copy(out=w2_sb[:, ic : ic + 1], in_=w2_ps[:])
            b2_ps = ps_at.tile([1, 1], F32, tag="at")
            for jc in range(KC):
                nc.tensor.matmul(
                    out=b2_ps[:],
                    lhsT=cwT[:, jc : jc + 1],
                    rhs=fbT[:, jc : jc + 1],
                    start=(jc == 0),
                    stop=(jc == KC - 1),
                )
            nc.vector.tensor_add(out=b2_sb[:], in0=b2_ps[:], in1=cb_sb[:])

        def emit_attention_own():
            with tc.tile_pool(name="attn_t", bufs=1) as pat:
                qkT_sb = pat.tile([P, KC, 2, P], F16)
                k_sb = pat.tile([P, D], F16)  # (s, i)
                xT_sb = pat.tile([P, D], F32)  # (i, (ic t))
                attn_b = pat.tile([P, NS], F32)  # (t, s)
                a_own = pat.tile([P, 1], F32)

                for ic in range(KC):
                    qkT_ps = ps_at.tile([P, 2 * P], F32, tag="at")
                    for kc in range(KC):
                        nc.tensor.matmul(
                            out=qkT_ps[:],
                            lhsT=qwT[:, kc, ic * P : (ic + 1) * P],
                            rhs=hh_own[:, kc],
                            start=(kc == 0),
                            stop=False,
                        )
                    nc.tensor.matmul(
                        out=qkT_ps[:],
                        lhsT=qb16[:, ic * P : (ic + 1) * P],
                        rhs=ones16[:],
                        start=False,
                        stop=True,
                    )
                    nc.vector.tensor_copy(
                        out=qkT_sb[:, ic],
                        in_=qkT_ps[:].rearrange("i (w t) -> i w t", t=P),
                    )

                k_ps = ps_at.tile([P, D], F32, tag="at")
                for kc in range(KC):
                    nc.tensor.matmul(
                        out=k_ps[:],
                        lhsT=hh_own[:, kc, 1, :],
                        rhs=qwT[:, kc, :],
                        start=(kc == 0),
                        stop=False,
                    )
                nc.tensor.matmul(
                    out=k_ps[:],
                    lhsT=ones16[:, 0:P],
                    rhs=qb16[:],
                    start=False,
                    stop=True,
                )
                nc.vector.tensor_copy(out=k_sb[:], in_=k_ps[:])

                s_ps = ps_at.tile([P, P], F32, tag="at")
                for ic in range(KC):
                    nc.tensor.matmul(
                        out=s_ps[:],
                        lhsT=qkT_sb[:, ic, 0, :],
                        rhs=qkT_sb[:, ic, 1, :],
                        start=(ic == 0),
                        stop=(ic == KC - 1),
                    )
                m_col = psm.tile([P, 1], F32, tag="m")
                negm = psm.tile([P, 1], F32, tag="negm")
                zatt = psm.tile([P, 1], F32, tag="zatt")
                rz = psm.tile([P, 1], F32, tag="rz")
                nc.vector.reduce_max(
                    out=m_col[:], in_=s_ps[:], axis=mybir.AxisListType.X
                )
                nc.vector.tensor_scalar_mul(negm[:], m_col[:], -INV_SQRT_D)
                nc.scalar.activation(
                    out=attn_b[:],
                    in_=s_ps[:],
                    func=AF.Exp,
                    bias=negm[:],
                    scale=INV_SQRT_D,
                    accum_out=zatt[:],
                )
                nc.vector.reciprocal(rz[:], zatt[:])
                nc.vector.tensor_scalar_mul(attn_b[:], attn_b[:], rz[:])

                t_ps = ps_at.tile([P, P], F32, tag="at")
                nc.tensor.transpose(t_ps[:], attn_b[:], identity[:])
                nc.vector.tensor_copy(out=ag_pack[:, 0:NT], in_=t_ps[:])

                x_ps = ps_at.tile([P, D], F32, tag="at")
                for ic in range(KC):
                    nc.tensor.matmul(
                        out=x_ps[:, ic * P : (ic + 1) * P],
                        lhsT=k_sb[:, ic * P : (ic + 1) * P],
                        rhs=ag_pack[:, 0:NT],
                        start=True,
                        stop=True,
                    )
                nc.vector.tensor_copy(out=xT_sb[:], in_=x_ps[:])

                c_ps = ps_at.tile([P, 1], F32, tag="at")
                for ic in range(KC):
                    nc.tensor.matmul(
                        out=c_ps[:],
                        lhsT=xT_sb[:, ic * P : (ic + 1) * P],
                        rhs=w2_sb[:, ic : ic + 1],
                        start=(ic == 0),
                        stop=False,
                    )
                nc.tensor.matmul(
                    out=c_ps[:],
                    lhsT=ones32[:],
                    rhs=b2_sb[:],
                    start=False,
                    stop=True,
                )
                nc.scalar.activation(out=a_own[:], in_=c_ps[:], func=AF.Sigmoid)
                nc.vector.tensor_copy(
                    out=ag_pack[:, NT : NT + 2].bitcast(F32), in_=a_own[:]
                )

            nc.sync.dma_start(out=ag_in[:], in_=ag_pack[:])
            nc.gpsimd.collective_compute(
                "AllGather",
                ALU.bypass,
                replica_groups=[list(range(NCORES))],
                ins=[ag_in[:].opt()],
                outs=[ag_out[:].opt()],
            )

        # ---- pass 1 + Z per group; pass 2 pipelined against next group ----
        with (
            tc.tile_pool(name="e", bufs=6) as pe,
            tc.tile_pool(name="io", bufs=3) as pio,
        ):
            e_tiles = {}

            def emit_pass1_batch(b):
                e_tiles[b] = pe.tile([P, VS], F32, tag="e", name=f"e_{b}")
                for n in range(NCH):
                    g_ps = ps_gen.tile([P, CH], F32, tag="g")
                    for kc in range(KC):
                        nc.tensor.matmul(
                            out=g_ps[:],
                            lhsT=htgtT[:, kc, b, :],
                            rhs=embT[:, kc, n * CH : (n + 1) * CH],
                            start=(kc == 0),
                            stop=(kc == KC - 1),
                        )
                    nc.scalar.activation(
                        out=e_tiles[b][:, n * CH : (n + 1) * CH],
                        in_=g_ps[:],
                        func=AF.Exp,
                        accum_out=zparts[:, b, n : n + 1],
                    )

            def emit_group_z(g):
                gs = slice(g * GB, (g + 1) * GB)
                nc.vector.reduce_sum(
                    out=zloc[:, gs], in_=zparts[:, gs, :], axis=mybir.AxisListType.X
                )
                nc.sync.dma_start(out=zin[g][:], in_=zloc[:, gs])
                nc.gpsimd.collective_compute(
                    "AllReduce",
                    ALU.add,
                    replica_groups=[list(range(NCORES))],
                    ins=[zin[g][:].opt()],
                    outs=[zout[g][:].opt()],
                )
                nc.sync.dma_start(out=zg_sb[:, gs], in_=zout[g][:])

            def emit_unpack():
                ag_r = ag_out[:].rearrange("(b s) w -> s b w", s=P)
                nc.sync.dma_start(out=attnT_all[:], in_=ag_r[:, :, 0:NT])
                nc.sync.dma_start(
                    out=a_all[:],
                    in_=ag_r[:, :, NT : NT + 2].bitcast(F32).squeeze(),
                )

            def emit_pass2_batch(b):
                oma = psm.tile([P, 1], F32, tag="oma")
                roma = psm.tile([P, 1], F32, tag="roma")
                rzg = psm.tile([P, 1], F32, tag="rzg")
                az = psm.tile([P, 1], F32, tag="az")
                c1_b = psm.tile([P, 1], F32, tag="c1")
                c2_b = psm.tile([P, 1], F32, tag="c2")
                nc.vector.tensor_scalar(
                    out=oma[:],
                    in0=a_all[:, b : b + 1],
                    scalar1=-1.0,
                    scalar2=1.0,
                    op0=ALU.mult,
                    op1=ALU.add,
                )
                nc.vector.reciprocal(roma[:], oma[:])
                nc.vector.reciprocal(rzg[:], zg_sb[:, b : b + 1])
                nc.vector.tensor_tensor(
                    out=c1_b[:], in0=oma[:], in1=rzg[:], op=ALU.mult
                )
                nc.vector.tensor_tensor(
                    out=az[:],
                    in0=a_all[:, b : b + 1],
                    in1=zg_sb[:, b : b + 1],
                    op=ALU.mult,
                )
                nc.vector.tensor_tensor(
                    out=c2_b[:], in0=az[:], in1=roma[:], op=ALU.mult
                )
                e_b = e_tiles[b]
                for n in range(NCH):
                    onehot = pio.tile([P, CH], F16, tag="oh")
                    nc.gpsimd.tensor_scalar(
                        out=onehot[:],
                        in0=iota_all[:, n, :],
                        scalar1=src_sb[:, b : b + 1],
                        scalar2=None,
                        op0=ALU.is_equal,
                    )
                    cp_ps = ps_gen.tile([P, CH], F32, tag="g")
                    nc.tensor.matmul(
                        out=cp_ps[:],
                        lhsT=attnT_all[:, b, :],
                        rhs=onehot[:],
                        start=True,
                        stop=True,
                    )
                    blend = pio.tile([P, CH], F32, tag="blend")
                    nc.vector.scalar_tensor_tensor(
                        out=blend[:],
                        in0=cp_ps[:],
                        scalar=c2_b[:],
                        in1=e_b[:, n * CH : (n + 1) * CH],
                        op0=ALU.mult,
                        op1=ALU.add,
                    )
                    half, hn = divmod(n, NCH // 2)
                    if hn == 0:
                        outt = pio.tile([P, VS // 2], F32, tag="outt")
                    nc.scalar.activation(
                        out=outt[:, hn * CH : (hn + 1) * CH],
                        in_=blend[:],
                        func=AF.Ln,
                        scale=c1_b[:],
                    )
                    if hn == NCH // 2 - 1:
                        nc.sync.dma_start(
                            out=out[:, b, half * (VS // 2) : (half + 1) * (VS // 2)],
                            in_=outt[:],
                        )

            # group 0 pass 1 (dense PE), attention + allgather slotted after
            emit_attention_own()
            for b in range(GB):
                emit_pass1_batch(b)
            emit_group_z(0)
            if ablate == "pass1g0":
                nc.sync.dma_start(out=out[0:1, 0, 0:4], in_=zparts[0:1, 0, 0:4])
                return
            emit_unpack()
            # interleave: pass1 of group 1 with pass 2 of group 0
            for i in range(GB):
                emit_pass1_batch(GB + i)
                emit_pass2_batch(i)
            emit_group_z(1)
            if ablate == "pass1":
                return
            for i in range(GB):
                emit_pass2_batch(GB + i)


_NC_CACHE = []


def _get_nc():
    if not _NC_CACHE:
        _NC_CACHE.append(build_kernel())
    return _NC_CACHE[0]


def _make_in_maps(inputs):
    htgt = np.ascontiguousarray(np.asarray(inputs["htgt"], dtype=np.float32))
    hsrc = np.ascontiguousarray(np.asarray(inputs["hsrc"], dtype=np.float32))
    src = np.ascontiguousarray(np.asarray(inputs["src"]).astype(np.int64))
    emb = np.ascontiguousarray(np.asarray(inputs["emb_weight"], dtype=np.float32))
    q_w = np.ascontiguousarray(np.asarray(inputs["q_w"], dtype=np.float32))
    q_b = np.ascontiguousarray(np.asarray(inputs["q_b"], dtype=np.float32))
    f_w = np.ascontiguousarray(np.asarray(inputs["f_w"], dtype=np.float32))
    f_b = np.ascontiguousarray(np.asarray(inputs["f_b"], dtype=np.float32))
    copy_w = np.ascontiguousarray(np.asarray(inputs["copy_w"], dtype=np.float32))
    copy_b = np.ascontiguousarray(np.asarray(inputs["copy_b"], dtype=np.float32))

    in_maps = []
    for c in range(NCORES):
        # integral values, exact in fp32 (scalar operand of is_equal must be f32)
        src_local = (src - c * VS).astype(np.float32)
        in_maps.append(
            {
                "htgt": htgt,
                "htgt_own": np.ascontiguousarray(htgt[:, c, :]),
                "hsrc_own": np.ascontiguousarray(hsrc[:, c, :]),
                "src_local": np.ascontiguousarray(src_local),
                "emb": np.ascontiguousarray(emb[c * VS : (c + 1) * VS]),
                "q_w": q_w,
                "q_b": q_b,
                "f_w": f_w,
                "f_b": f_b,
                "copy_w": copy_w,
                "copy_b": copy_b,
            }
        )
    return in_maps


def kernel(**inputs):
    in_maps = _make_in_maps(inputs)
    nc = _get_nc()
    res = run_bass_kernel_spmd(nc, in_maps, list(range(NCORES))).results
    return np.concatenate([res[c]["out"] for c in range(NCORES)], axis=2)



# revision 2
# speedup vs baseline: 3.4384x; 3.4384x over previous
"""CopyGenerator kernel for Trainium2 (Bass/Tile), vocab-parallel over 8 cores.

Per core c (vocab shard [c*4000, (c+1)*4000), attention batch c):
  attention for OWN batch only -> attnT_own, a_own; AllGather (33KB) shares
  all batches' attnT/a with every core (latency hidden under pass 1).
  gen_score = htgt @ emb_shard.T                       (PE, fp16)
  e = exp(gen_score)   [no max-sub; scores are O(3)]   (ACT, fused row-sum)
  Z = allreduce_add(sum_v e), split into two batch groups so pass 2 of
      group 0 overlaps pass 1 of group 1.
  copy_p shard = attn @ onehot(src_local)              (PE, fp16 exact onehot)
  out = log(a*copy_p + (1-a)*e/Z) = Ln(c1*(c2*copy_p + e)),
      c1=(1-a)/Z, c2=a*Z/(1-a)

All transposed operands (embT, htgtT, hh_own, qwT) are pre-transposed and
cast to fp16 on the host, so they DMA straight into SBUF with no PE
transposes.  The vocab shard is zero-padded to 4096 so PSUM chunks are
bank-aligned (512 cols); the pad contributes exp(0)=1 per column to Z,
subtracted as a constant.  The one-hot is built on the Vector engine
(is_equal against an i16 iota) — GpSimd is ~20x slower there.
"""

import sys

sys.path.insert(0, "/opt/trn_rl_repo")

import numpy as np

from concourse import bass, bacc, mybir
import concourse.tile as tile
from concourse.bass_utils import run_bass_kernel_spmd
from concourse.masks import make_identity

NT, NS, BS, D, V = 128, 128, 8, 512, 32000
NCORES = 8
VS = V // NCORES  # 4000 vocab per core
VSP = 4096  # padded (bank-aligned) vocab per core
NPAIR = 4  # 1024-col (2 PSUM bank) pairs per batch
PW = VSP // NPAIR  # 1024
CH = 512  # cols per PSUM bank
P = 128
KC = D // P  # 4 contraction chunks
NG = 2  # Z-collective batch groups
GB = BS // NG  # batches per group
F32 = mybir.dt.float32
F16 = mybir.dt.float16
I16 = mybir.dt.int16
AF = mybir.ActivationFunctionType
ALU = mybir.AluOpType
INV_SQRT_D = 1.0 / float(np.sqrt(np.float32(D)))
AGW = NT + 2  # allgather row width: attnT row (t) + a (1 fp32 = 2 fp16)
NPAD = VSP - VS  # 96 pad columns -> exp(0)=1 each, subtracted from Z


def build_kernel():
    nc = bacc.Bacc(
        "TRN2",
        target_bir_lowering=False,
        debug=False,
        enable_asserts=False,
        num_devices=NCORES,
    )
    embT_h = nc.dram_tensor("embT_h", [KC, P, VSP], F16, kind="ExternalInput").ap()
    htgtT_h = nc.dram_tensor("htgtT_h", [KC, P, BS, P], F16, kind="ExternalInput").ap()
    hh_h = nc.dram_tensor("hh_h", [KC, P, 2, P], F16, kind="ExternalInput").ap()
    qwT_h = nc.dram_tensor("qwT_h", [KC, P, D], F16, kind="ExternalInput").ap()
    qb_h = nc.dram_tensor("qb_h", [1, D], F16, kind="ExternalInput").ap()
    src_h = nc.dram_tensor("src_h", [NS, BS], F32, kind="ExternalInput").ap()
    w2_h = nc.dram_tensor("w2_h", [P, KC], F32, kind="ExternalInput").ap()
    b2_h = nc.dram_tensor("b2_h", [1, 1], F32, kind="ExternalInput").ap()
    out = nc.dram_tensor("out", [NT, BS, VS], F32, kind="ExternalOutput").ap()

    with tile.TileContext(nc) as tc:
        _emit(nc, tc, embT_h, htgtT_h, hh_h, qwT_h, qb_h, src_h, w2_h, b2_h, out)
    nc.compile()
    return nc


def _emit(nc, tc, embT_h, htgtT_h, hh_h, qwT_h, qb_h, src_h, w2_h, b2_h, out):
    with (
        tc.tile_pool(name="persist", bufs=1) as pw,
        tc.tile_pool(name="small", bufs=2) as psm,
        tc.tile_pool(name="ps_gen", bufs=2, space="PSUM") as ps_gen,
        tc.tile_pool(name="ps_cp", bufs=2, space="PSUM") as ps_cp,
        tc.tile_pool(name="dram", bufs=1, space="DRAM") as pdram,
    ):
        # ---- persistent SBUF ----
        embT = pw.tile([P, KC, VSP], F16)  # (d, kc, v)
        htgtT = pw.tile([P, KC, BS, P], F16)  # (d, kc, b, t)
        hh_own = pw.tile([P, KC, 2, P], F16)  # (d, kc, {tgt,src}, t/s)
        qwT = pw.tile([P, KC, D], F16)  # (d, kc, i)
        qb16 = pw.tile([1, D], F16)
        attnT_all = pw.tile([P, BS, NT], F16)  # (s, b, t)
        a_all = pw.tile([P, BS], F32)
        src_sb = pw.tile([P, BS], F32)
        iota_all = pw.tile([P, VSP], I16)
        w2_sb = pw.tile([P, KC], F32)
        b2_sb = pw.tile([1, 1], F32)
        identity = pw.tile([P, P], F32)
        ones16 = pw.tile([1, 2 * P], F16)
        ones32 = pw.tile([1, P], F32)
        zparts = pw.tile([P, BS, NPAIR], F32)
        zloc = pw.tile([P, BS], F32)
        zg_sb = pw.tile([P, BS], F32)
        ag_pack = pw.tile([P, AGW], F16)  # (s, t | a-bits)

        ag_in = pdram.tile([P, AGW], F16)
        ag_out = pdram.tile([NCORES * P, AGW], F16)
        zin = [pdram.tile([P, GB], F32, name=f"zin{g}") for g in range(NG)]
        zout = [pdram.tile([P, GB], F32, name=f"zout{g}") for g in range(NG)]

        # ---- loads (small attention-critical ones first) ----
        nc.sync.dma_start(out=src_sb[:], in_=src_h)
        nc.sync.dma_start(out=w2_sb[:], in_=w2_h)
        nc.sync.dma_start(out=b2_sb[:], in_=b2_h)
        nc.sync.dma_start(out=qb16[:], in_=qb_h)
        for kc in range(KC):
            nc.sync.dma_start(out=hh_own[:, kc], in_=hh_h[kc])
        for kc in range(KC):
            nc.sync.dma_start(out=qwT[:, kc], in_=qwT_h[kc])
        for kc in range(KC):
            nc.sync.dma_start(out=embT[:, kc], in_=embT_h[kc])
        for kc in range(KC):
            nc.sync.dma_start(out=htgtT[:, kc], in_=htgtT_h[kc])

        make_identity(nc, identity[:])
        nc.vector.memset(ones16[:], 1.0)
        nc.vector.memset(ones32[:], 1.0)
        for n in range(8):
            nc.gpsimd.iota(
                iota_all[:, n * CH : (n + 1) * CH],
                pattern=[[1, CH]],
                base=n * CH,
                channel_multiplier=0,
            )

        def emit_attention_own():
            with tc.tile_pool(name="attn_t", bufs=1) as pat:
                qkT_sb = pat.tile([P, KC, 2, P], F16)
                k_sb = pat.tile([P, D], F16)  # (s, i)
                xT_sb = pat.tile([P, D], F32)  # (i, (ic t))
                attn_b = pat.tile([P, NS], F32)  # (t, s)
                a_own = pat.tile([P, 1], F32)

                for ic in range(KC):
                    qkT_ps = ps_cp.tile([P, PW], F32, tag="c")
                    for kc in range(KC):
                        nc.tensor.matmul(
                            out=qkT_ps[:, 0 : 2 * P],
                            lhsT=qwT[:, kc, ic * P : (ic + 1) * P],
                            rhs=hh_own[:, kc],
                            start=(kc == 0),
                            stop=False,
                        )
                    nc.tensor.matmul(
                        out=qkT_ps[:, 0 : 2 * P],
                        lhsT=qb16[:, ic * P : (ic + 1) * P],
                        rhs=ones16[:],
                        start=False,
                        stop=True,
                    )
                    nc.vector.tensor_copy(
                        out=qkT_sb[:, ic],
                        in_=qkT_ps[:, 0 : 2 * P].rearrange("i (w t) -> i w t", t=P),
                    )

                k_ps = ps_cp.tile([P, PW], F32, tag="c")
                for kc in range(KC):
                    nc.tensor.matmul(
                        out=k_ps[:, 0:D],
                        lhsT=hh_own[:, kc, 1, :],
                        rhs=qwT[:, kc, :],
                        start=(kc == 0),
                        stop=False,
                    )
                nc.tensor.matmul(
                    out=k_ps[:, 0:D],
                    lhsT=ones16[:, 0:P],
                    rhs=qb16[:],
                    start=False,
                    stop=True,
                )
                nc.vector.tensor_copy(out=k_sb[:], in_=k_ps[:, 0:D])

                s_ps = ps_cp.tile([P, PW], F32, tag="c")
                for ic in range(KC):
                    nc.tensor.matmul(
                        out=s_ps[:, 0:P],
                        lhsT=qkT_sb[:, ic, 0, :],
                        rhs=qkT_sb[:, ic, 1, :],
                        start=(ic == 0),
                        stop=(ic == KC - 1),
                    )
                m_col = psm.tile([P, 1], F32, tag="m")
                negm = psm.tile([P, 1], F32, tag="negm")
                zatt = psm.tile([P, 1], F32, tag="zatt")
                rz = psm.tile([P, 1], F32, tag="rz")
                nc.vector.reduce_max(
                    out=m_col[:], in_=s_ps[:, 0:P], axis=mybir.AxisListType.X
                )
                nc.vector.tensor_scalar_mul(negm[:], m_col[:], -INV_SQRT_D)
                nc.scalar.activation(
                    out=attn_b[:],
                    in_=s_ps[:, 0:P],
                    func=AF.Exp,
                    bias=negm[:],
                    scale=INV_SQRT_D,
                    accum_out=zatt[:],
                )
                nc.vector.reciprocal(rz[:], zatt[:])
                nc.vector.tensor_scalar_mul(attn_b[:], attn_b[:], rz[:])

                t_ps = ps_cp.tile([P, PW], F32, tag="c")
                nc.tensor.transpose(t_ps[:, 0:P], attn_b[:], identity[:])
                nc.vector.tensor_copy(out=ag_pack[:, 0:NT], in_=t_ps[:, 0:P])

                x_ps = ps_cp.tile([P, PW], F32, tag="c")
                for ic in range(KC):
                    nc.tensor.matmul(
                        out=x_ps[:, ic * P : (ic + 1) * P],
                        lhsT=k_sb[:, ic * P : (ic + 1) * P],
                        rhs=ag_pack[:, 0:NT],
                        start=True,
                        stop=True,
                    )
                nc.vector.tensor_copy(out=xT_sb[:], in_=x_ps[:, 0:D])

                c_ps = ps_cp.tile([P, PW], F32, tag="c")
                for ic in range(KC):
                    nc.tensor.matmul(
                        out=c_ps[:, 0:1],
                        lhsT=xT_sb[:, ic * P : (ic + 1) * P],
                        rhs=w2_sb[:, ic : ic + 1],
                        start=(ic == 0),
                        stop=False,
                    )
                nc.tensor.matmul(
                    out=c_ps[:, 0:1],
                    lhsT=ones32[:],
                    rhs=b2_sb[:],
                    start=False,
                    stop=True,
                )
                # a = sigmoid(c) = 1/(1+exp(-c)); avoids the sigmoid ACT
                # table set (exp+ln share natural_log_exp_and_others)
                ec = psm.tile([P, 1], F32, tag="ec")
                den = psm.tile([P, 1], F32, tag="den")
                nc.scalar.activation(
                    out=ec[:], in_=c_ps[:, 0:1], func=AF.Exp, scale=-1.0
                )
                nc.vector.tensor_scalar_add(den[:], ec[:], 1.0)
                nc.vector.reciprocal(a_own[:], den[:])
                nc.vector.tensor_copy(
                    out=ag_pack[:, NT : NT + 2].bitcast(F32), in_=a_own[:]
                )

            nc.sync.dma_start(out=ag_in[:], in_=ag_pack[:])
            nc.gpsimd.collective_compute(
                "AllGather",
                ALU.bypass,
                replica_groups=[list(range(NCORES))],
                ins=[ag_in[:].opt()],
                outs=[ag_out[:].opt()],
            )

        # ---- pass 1 + Z per group; pass 2 pipelined against next group ----
        with (
            tc.tile_pool(name="e", bufs=6) as pe,
            tc.tile_pool(name="oh", bufs=2) as poh,
            tc.tile_pool(name="bl", bufs=2) as pbl,
            tc.tile_pool(name="ot", bufs=2) as pot,
        ):
            e_tiles = {}

            def emit_pass1_batch(b):
                e_tiles[b] = pe.tile([P, VSP], F16, tag="e", name=f"e_{b}")
                for p in range(NPAIR):
                    g_ps = ps_gen.tile([P, PW], F32, tag="g")
                    for kc in range(KC):
                        nc.tensor.matmul(
                            out=g_ps[:, 0:CH],
                            lhsT=htgtT[:, kc, b, :],
                            rhs=embT[:, kc, p * PW : p * PW + CH],
                            start=(kc == 0),
                            stop=(kc == KC - 1),
                        )
                        nc.tensor.matmul(
                            out=g_ps[:, CH:PW],
                            lhsT=htgtT[:, kc, b, :],
                            rhs=embT[:, kc, p * PW + CH : (p + 1) * PW],
                            start=(kc == 0),
                            stop=(kc == KC - 1),
                        )
                    nc.scalar.activation(
                        out=e_tiles[b][:, p * PW : (p + 1) * PW],
                        in_=g_ps[:],
                        func=AF.Exp,
                        accum_out=zparts[:, b, p : p + 1],
                    )

            def emit_group_z(g):
                gs = slice(g * GB, (g + 1) * GB)
                nc.vector.reduce_sum(
                    out=zloc[:, gs], in_=zparts[:, gs, :], axis=mybir.AxisListType.X
                )
                # remove the VSP-VS zero-pad columns' exp(0)=1 contributions
                nc.vector.tensor_scalar_add(zloc[:, gs], zloc[:, gs], -float(NPAD))
                nc.sync.dma_start(out=zin[g][:], in_=zloc[:, gs])
                nc.gpsimd.collective_compute(
                    "AllReduce",
                    ALU.add,
                    replica_groups=[list(range(NCORES))],
                    ins=[zin[g][:].opt()],
                    outs=[zout[g][:].opt()],
                )
                nc.sync.dma_start(out=zg_sb[:, gs], in_=zout[g][:])

            def emit_unpack():
                ag_r = ag_out[:].rearrange("(b s) w -> s b w", s=P)
                nc.sync.dma_start(out=attnT_all[:], in_=ag_r[:, :, 0:NT])
                nc.sync.dma_start(
                    out=a_all[:],
                    in_=ag_r[:, :, NT : NT + 2].bitcast(F32).squeeze(),
                )

            def emit_pass2_batch(b):
                oma = psm.tile([P, 1], F32, tag="oma")
                roma = psm.tile([P, 1], F32, tag="roma")
                rzg = psm.tile([P, 1], F32, tag="rzg")
                az = psm.tile([P, 1], F32, tag="az")
                c1_b = psm.tile([P, 1], F32, tag="c1")
                c2_b = psm.tile([P, 1], F32, tag="c2")
                nc.vector.tensor_scalar(
                    out=oma[:],
                    in0=a_all[:, b : b + 1],
                    scalar1=-1.0,
                    scalar2=1.0,
                    op0=ALU.mult,
                    op1=ALU.add,
                )
                nc.vector.reciprocal(roma[:], oma[:])
                nc.vector.reciprocal(rzg[:], zg_sb[:, b : b + 1])
                nc.vector.tensor_tensor(
                    out=c1_b[:], in0=oma[:], in1=rzg[:], op=ALU.mult
                )
                nc.vector.tensor_tensor(
                    out=az[:],
                    in0=a_all[:, b : b + 1],
                    in1=zg_sb[:, b : b + 1],
                    op=ALU.mult,
                )
                nc.vector.tensor_tensor(
                    out=c2_b[:], in0=az[:], in1=roma[:], op=ALU.mult
                )
                onehot = poh.tile([P, VSP], F16, tag="oh")
                nc.vector.tensor_scalar(
                    out=onehot[:],
                    in0=iota_all[:],
                    scalar1=src_sb[:, b : b + 1],
                    scalar2=None,
                    op0=ALU.is_equal,
                )
                e_b = e_tiles[b]
                blend = pbl.tile([P, VSP], F32, tag="bl")
                for p in range(NPAIR):
                    cp_ps = ps_cp.tile([P, PW], F32, tag="c")
                    nc.tensor.matmul(
                        out=cp_ps[:, 0:CH],
                        lhsT=attnT_all[:, b, :],
                        rhs=onehot[:, p * PW : p * PW + CH],
                        start=True,
                        stop=True,
                    )
                    nc.tensor.matmul(
                        out=cp_ps[:, CH:PW],
                        lhsT=attnT_all[:, b, :],
                        rhs=onehot[:, p * PW + CH : (p + 1) * PW],
                        start=True,
                        stop=True,
                    )
                    nc.vector.scalar_tensor_tensor(
                        out=blend[:, p * PW : (p + 1) * PW],
                        in0=cp_ps[:],
                        scalar=c2_b[:],
                        in1=e_b[:, p * PW : (p + 1) * PW],
                        op0=ALU.mult,
                        op1=ALU.add,
                    )
                outt = pot.tile([P, VSP], F32, tag="ot")
                nc.scalar.activation(
                    out=outt[:], in_=blend[:], func=AF.Ln, scale=c1_b[:]
                )
                nc.sync.dma_start(out=out[:, b, :], in_=outt[:, 0:VS])

            # group 0 pass 1 (dense PE), attention + allgather slotted after
            emit_attention_own()
            for b in range(GB):
                emit_pass1_batch(b)
            emit_group_z(0)
            emit_unpack()
            # interleave: pass1 of group 1 with pass 2 of group 0
            for i in range(GB):
                emit_pass1_batch(GB + i)
                emit_pass2_batch(i)
            emit_group_z(1)
            for i in range(GB):
                emit_pass2_batch(GB + i)


_NC_CACHE = []


def _get_nc():
    if not _NC_CACHE:
        _NC_CACHE.append(build_kernel())
    return _NC_CACHE[0]


def _make_in_maps(inputs):
    htgt = np.asarray(inputs["htgt"], dtype=np.float32)
    hsrc = np.asarray(inputs["hsrc"], dtype=np.float32)
    src = np.asarray(inputs["src"]).astype(np.int64)
    emb = np.asarray(inputs["emb_weight"], dtype=np.float32)
    q_w = np.asarray(inputs["q_w"], dtype=np.float32)
    q_b = np.asarray(inputs["q_b"], dtype=np.float32)
    f_w = np.asarray(inputs["f_w"], dtype=np.float32)
    f_b = np.asarray(inputs["f_b"], dtype=np.float32)
    copy_w = np.asarray(inputs["copy_w"], dtype=np.float32)
    copy_b = np.asarray(inputs["copy_b"], dtype=np.float32)

    # shared across cores
    htgtT_h = np.ascontiguousarray(
        htgt.transpose(2, 1, 0).astype(np.float16).reshape(KC, P, BS, P)
    )
    qwT_h = np.ascontiguousarray(q_w.T).astype(np.float16).reshape(KC, P, D)
    qb_h = np.ascontiguousarray(q_b.astype(np.float16).reshape(1, D))
    # fold f_w/copy_w:  w2 = f_w.T @ copy_w.T ;  b2 = copy_w @ f_b + copy_b
    w2_full = f_w.T @ copy_w[0]  # [D]
    w2_h = np.ascontiguousarray(w2_full.reshape(KC, P).T)  # [P, KC]
    b2_h = np.ascontiguousarray(
        (copy_w[0] @ f_b + copy_b[0]).reshape(1, 1).astype(np.float32)
    )

    in_maps = []
    for c in range(NCORES):
        eT = np.zeros((D, VSP), dtype=np.float16)
        eT[:, 0:VS] = emb[c * VS : (c + 1) * VS].T.astype(np.float16)
        embT_h = np.ascontiguousarray(eT.reshape(KC, P, VSP))
        hh_h = np.ascontiguousarray(
            np.stack([htgt[:, c, :].T, hsrc[:, c, :].T], axis=1)
            .astype(np.float16)
            .reshape(KC, P, 2, P)
        )
        # integral values, exact in fp32 (scalar operand of is_equal is f32)
        src_local = np.ascontiguousarray((src - c * VS).astype(np.float32))
        in_maps.append(
            {
                "embT_h": embT_h,
                "htgtT_h": htgtT_h,
                "hh_h": hh_h,
                "qwT_h": qwT_h,
                "qb_h": qb_h,
                "src_h": src_local,
                "w2_h": w2_h,
                "b2_h": b2_h,
            }
        )
    return in_maps


def kernel(**inputs):
    in_maps = _make_in_maps(inputs)
    nc = _get_nc()
    res = run_bass_kernel_spmd(nc, in_maps, list(range(NCORES))).results
    return np.concatenate([res[c]["out"] for c in range(NCORES)], axis=2)


# revision 3
# speedup vs baseline: 3.5454x; 1.0311x over previous
"""CopyGenerator kernel for Trainium2 (Bass/Tile), vocab-parallel over 8 cores.

Per core c (vocab shard [c*4000, (c+1)*4000), attention batch c):
  attention for OWN batch only -> attnT_own, a_own; AllGather (33KB) shares
  all batches' attnT/a with every core (latency hidden under pass 1).
  pass 1 (all 8 batches): gen = htgt @ emb_shard.T    (PE, fp16)
    e = exp(gen)  [no max-sub; scores are O(3)]       (ACT)
    Z_local = rowsum(e)                               (DVE)
    Z = allreduce_add(Z_local), two batch groups so the first AllReduce
    overlaps the second half of pass 1.
  pass 2 (all 8 batches): psum = K*copy_p + K*rc2 * e, computed entirely
    on the PE: attnT @ (K*onehot(src)) accumulated with diag(K*rc2) @ e,
    where rc2 = (1-a)/(a*Z) and K=128 keeps diag entries in fp16 normal
    range.  out = Ln((a/K) * psum) reads PSUM directly.
      check: (a/K)*(K*cp + K*rc2*e) = a*cp + (1-a)*e/Z  -> log blend. ✓

All transposed operands (embT, htgtT, hh_own, qwT) are pre-transposed and
cast to fp16 on the host, so they DMA straight into SBUF with no PE
transposes.  The vocab shard is zero-padded to 4096 so PSUM chunks are
bank-aligned; pad columns contribute exp(0)=1 each to Z, subtracted as a
constant, and are never DMA'd to the output.  The one-hot (values K) is
built on the Vector engine (is_equal + mult in one op) — GpSimd is ~20x
slower there.  All pass-1 work is emitted before any pass-2 work so the
in-order PE queue never head-of-line blocks on the AllGather, keeping the
PE HAM-warm, and so exp/ln activations don't thrash ACT table sets.
"""

import sys

sys.path.insert(0, "/opt/trn_rl_repo")

import numpy as np

from concourse import bass, bacc, mybir
import concourse.tile as tile
from concourse.bass_utils import run_bass_kernel_spmd
from concourse.masks import make_identity

NT, NS, BS, D, V = 128, 128, 8, 512, 32000
NCORES = 8
VS = V // NCORES  # 4000 vocab per core
VSP = 4096  # padded (bank-aligned) vocab per core
NPAIR = 4  # 1024-col (2 PSUM bank) pairs per batch
PW = VSP // NPAIR  # 1024
CH = 512  # cols per PSUM bank
P = 128
KC = D // P  # 4 contraction chunks
NG = 2  # Z-collective batch groups
GB = BS // NG  # batches per group
K = 128.0  # onehot scale: keeps diag(K*rc2) in fp16 normal range
F32 = mybir.dt.float32
F16 = mybir.dt.float16
I16 = mybir.dt.int16
AF = mybir.ActivationFunctionType
ALU = mybir.AluOpType
INV_SQRT_D = 1.0 / float(np.sqrt(np.float32(D)))
AGW = NT + 2  # allgather row width: attnT row (t) + a (1 fp32 = 2 fp16)
NPAD = VSP - VS  # 96 pad columns -> exp(0)=1 each, subtracted from Z


def build_kernel():
    nc = bacc.Bacc(
        "TRN2",
        target_bir_lowering=False,
        debug=False,
        enable_asserts=False,
        num_devices=NCORES,
    )
    embT_h = nc.dram_tensor("embT_h", [KC, P, VSP], F16, kind="ExternalInput").ap()
    htgtT_h = nc.dram_tensor("htgtT_h", [KC, P, BS, P], F16, kind="ExternalInput").ap()
    hh_h = nc.dram_tensor("hh_h", [KC, P, 2, P], F16, kind="ExternalInput").ap()
    qwT_h = nc.dram_tensor("qwT_h", [KC, P, D], F16, kind="ExternalInput").ap()
    qb_h = nc.dram_tensor("qb_h", [1, D], F16, kind="ExternalInput").ap()
    src_h = nc.dram_tensor("src_h", [NS, BS], F32, kind="ExternalInput").ap()
    w2_h = nc.dram_tensor("w2_h", [P, KC], F16, kind="ExternalInput").ap()
    nb2_h = nc.dram_tensor("nb2_h", [P, 1], F32, kind="ExternalInput").ap()
    out = nc.dram_tensor("out", [NT, BS, VS], F32, kind="ExternalOutput").ap()

    with tile.TileContext(nc) as tc:
        _emit(nc, tc, embT_h, htgtT_h, hh_h, qwT_h, qb_h, src_h, w2_h, nb2_h, out)
    nc.compile()
    return nc


def _emit(nc, tc, embT_h, htgtT_h, hh_h, qwT_h, qb_h, src_h, w2_h, nb2_h, out):
    with (
        tc.tile_pool(name="persist", bufs=1) as pw,
        tc.tile_pool(name="small", bufs=2) as psm,
        tc.tile_pool(name="ps_gen", bufs=2, space="PSUM") as ps_gen,
        tc.tile_pool(name="ps_cp", bufs=2, space="PSUM") as ps_cp,
        tc.tile_pool(name="dram", bufs=1, space="DRAM") as pdram,
    ):
        # ---- persistent SBUF ----
        embT = pw.tile([P, KC, VSP], F16)  # (d, kc, v)
        htgtT = pw.tile([P, KC, BS, P], F16)  # (d, kc, b, t)
        hh_own = pw.tile([P, KC, 2, P], F16)  # (d, kc, {tgt,src}, t/s)
        qwT = pw.tile([P, KC, D], F16)  # (d, kc, i)
        qb16 = pw.tile([1, D], F16)
        attnT_all = pw.tile([P, BS, NT], F16)  # (s, b, t)
        a_all = pw.tile([P, BS], F32)
        aK_all = pw.tile([P, BS], F32)  # a / K
        rc2K_all = pw.tile([P, BS], F32)  # K * (1-a) / (a*Z)
        src_sb = pw.tile([P, BS], F32)
        iota_all = pw.tile([P, VSP], I16)
        w2_sb = pw.tile([P, KC], F16)
        nb2_sb = pw.tile([P, 1], F32)
        identity = pw.tile([P, P], F32)
        ones16 = pw.tile([1, 2 * P], F16)
        zloc = pw.tile([P, BS], F32)
        zg_sb = pw.tile([P, BS], F32)
        ag_pack = pw.tile([P, AGW], F16)  # (s, t | a-bits)

        ag_in = pdram.tile([P, AGW], F16)
        ag_out = pdram.tile([NCORES * P, AGW], F16)
        zin = [pdram.tile([P, GB], F32, name=f"zin{g}") for g in range(NG)]
        zout = [pdram.tile([P, GB], F32, name=f"zout{g}") for g in range(NG)]

        # ---- loads (small attention-critical ones first) ----
        nc.sync.dma_start(out=src_sb[:], in_=src_h)
        nc.sync.dma_start(out=w2_sb[:], in_=w2_h)
        nc.sync.dma_start(out=nb2_sb[:], in_=nb2_h)
        nc.sync.dma_start(out=qb16[:], in_=qb_h)
        for kc in range(KC):
            nc.sync.dma_start(out=hh_own[:, kc], in_=hh_h[kc])
        for kc in range(KC):
            nc.sync.dma_start(out=qwT[:, kc], in_=qwT_h[kc])
        for kc in range(KC):
            nc.sync.dma_start(out=embT[:, kc], in_=embT_h[kc])
        for kc in range(KC):
            nc.sync.dma_start(out=htgtT[:, kc], in_=htgtT_h[kc])

        make_identity(nc, identity[:])
        nc.vector.memset(ones16[:], 1.0)
        for n in range(8):
            nc.gpsimd.iota(
                iota_all[:, n * CH : (n + 1) * CH],
                pattern=[[1, CH]],
                base=n * CH,
                channel_multiplier=0,
            )

        def emit_attention_own():
            with tc.tile_pool(name="attn_t", bufs=1) as pat:
                qkT_sb = pat.tile([P, KC, 2, P], F16)
                vw2_sb = pat.tile([P, 1], F16)  # (s, 1): v @ w2
                attn_b = pat.tile([P, NS], F32)  # (t, s)
                a_own = pat.tile([P, 1], F32)

                for ic in range(KC):
                    qkT_ps = ps_cp.tile([P, PW], F32, tag="c")
                    for kc in range(KC):
                        nc.tensor.matmul(
                            out=qkT_ps[:, 0 : 2 * P],
                            lhsT=qwT[:, kc, ic * P : (ic + 1) * P],
                            rhs=hh_own[:, kc],
                            start=(kc == 0),
                            stop=False,
                        )
                    nc.tensor.matmul(
                        out=qkT_ps[:, 0 : 2 * P],
                        lhsT=qb16[:, ic * P : (ic + 1) * P],
                        rhs=ones16[:],
                        start=False,
                        stop=True,
                    )
                    nc.vector.tensor_copy(
                        out=qkT_sb[:, ic],
                        in_=qkT_ps[:, 0 : 2 * P].rearrange("i (w t) -> i w t", t=P),
                    )

                # vw2[s] = sum_i k[s,i] * w2[i]  (v == k)
                vw2_ps = ps_cp.tile([P, PW], F32, tag="c")
                for ic in range(KC):
                    nc.tensor.matmul(
                        out=vw2_ps[:, 0:1],
                        lhsT=qkT_sb[:, ic, 1, :],
                        rhs=w2_sb[:, ic : ic + 1],
                        start=(ic == 0),
                        stop=(ic == KC - 1),
                    )
                nc.vector.tensor_copy(out=vw2_sb[:], in_=vw2_ps[:, 0:1])

                s_ps = ps_cp.tile([P, PW], F32, tag="c")
                for ic in range(KC):
                    nc.tensor.matmul(
                        out=s_ps[:, 0:P],
                        lhsT=qkT_sb[:, ic, 0, :],
                        rhs=qkT_sb[:, ic, 1, :],
                        start=(ic == 0),
                        stop=(ic == KC - 1),
                    )
                m_col = psm.tile([P, 1], F32, tag="m")
                negm = psm.tile([P, 1], F32, tag="negm")
                zatt = psm.tile([P, 1], F32, tag="zatt")
                rz = psm.tile([P, 1], F32, tag="rz")
                nc.vector.reduce_max(
                    out=m_col[:], in_=s_ps[:, 0:P], axis=mybir.AxisListType.X
                )
                nc.vector.tensor_scalar_mul(negm[:], m_col[:], -INV_SQRT_D)
                nc.scalar.activation(
                    out=attn_b[:],
                    in_=s_ps[:, 0:P],
                    func=AF.Exp,
                    bias=negm[:],
                    scale=INV_SQRT_D,
                    accum_out=zatt[:],
                )
                nc.vector.reciprocal(rz[:], zatt[:])
                nc.vector.tensor_scalar_mul(attn_b[:], attn_b[:], rz[:])

                t_ps = ps_cp.tile([P, PW], F32, tag="c")
                nc.tensor.transpose(t_ps[:, 0:P], attn_b[:], identity[:])
                nc.vector.tensor_copy(out=ag_pack[:, 0:NT], in_=t_ps[:, 0:P])

                # a = sigmoid(attn @ vw2 + b2), via exp (stays in one ACT set)
                c_ps = ps_cp.tile([P, PW], F32, tag="c")
                nc.tensor.matmul(
                    out=c_ps[:, 0:1],
                    lhsT=ag_pack[:, 0:NT],
                    rhs=vw2_sb[:],
                    start=True,
                    stop=True,
                )
                ec = psm.tile([P, 1], F32, tag="ec")
                den = psm.tile([P, 1], F32, tag="den")
                nc.scalar.activation(
                    out=ec[:], in_=c_ps[:, 0:1], func=AF.Exp, scale=-1.0,
                    bias=nb2_sb[:],
                )
                nc.vector.tensor_scalar_add(den[:], ec[:], 1.0)
                nc.vector.reciprocal(a_own[:], den[:])
                nc.vector.tensor_copy(
                    out=ag_pack[:, NT : NT + 2].bitcast(F32), in_=a_own[:]
                )

            nc.sync.dma_start(out=ag_in[:], in_=ag_pack[:])
            nc.gpsimd.collective_compute(
                "AllGather",
                ALU.bypass,
                replica_groups=[list(range(NCORES))],
                ins=[ag_in[:].opt()],
                outs=[ag_out[:].opt()],
            )

        with (
            tc.tile_pool(name="e", bufs=BS) as pe,
            tc.tile_pool(name="oh", bufs=2) as poh,
            tc.tile_pool(name="dg", bufs=2) as pdg,
            tc.tile_pool(name="ot", bufs=3) as pot,
        ):
            e_tiles = {}

            def emit_pass1_batch(b):
                e_tiles[b] = pe.tile([P, VSP], F16, tag="e", name=f"e_{b}")
                for p in range(NPAIR):
                    g_ps = ps_gen.tile([P, PW], F32, tag="g")
                    for kc in range(KC):
                        nc.tensor.matmul(
                            out=g_ps[:, 0:CH],
                            lhsT=htgtT[:, kc, b, :],
                            rhs=embT[:, kc, p * PW : p * PW + CH],
                            start=(kc == 0),
                            stop=(kc == KC - 1),
                        )
                        nc.tensor.matmul(
                            out=g_ps[:, CH:PW],
                            lhsT=htgtT[:, kc, b, :],
                            rhs=embT[:, kc, p * PW + CH : (p + 1) * PW],
                            start=(kc == 0),
                            stop=(kc == KC - 1),
                        )
                    nc.scalar.activation(
                        out=e_tiles[b][:, p * PW : (p + 1) * PW],
                        in_=g_ps[:],
                        func=AF.Exp,
                    )
                # Z_local row-sum on DVE (frees ACT accumulator reads)
                nc.vector.reduce_sum(
                    out=zloc[:, b : b + 1],
                    in_=e_tiles[b][:],
                    axis=mybir.AxisListType.X,
                )

            def emit_group_z(g):
                gs = slice(g * GB, (g + 1) * GB)
                # remove the VSP-VS zero-pad columns' exp(0)=1 contributions
                nc.vector.tensor_scalar_add(zloc[:, gs], zloc[:, gs], -float(NPAD))
                nc.sync.dma_start(out=zin[g][:], in_=zloc[:, gs])
                nc.gpsimd.collective_compute(
                    "AllReduce",
                    ALU.add,
                    replica_groups=[list(range(NCORES))],
                    ins=[zin[g][:].opt()],
                    outs=[zout[g][:].opt()],
                )
                nc.sync.dma_start(out=zg_sb[:, gs], in_=zout[g][:])

            def emit_unpack():
                ag_r = ag_out[:].rearrange("(b s) w -> s b w", s=P)
                nc.sync.dma_start(out=attnT_all[:], in_=ag_r[:, :, 0:NT])
                nc.sync.dma_start(
                    out=a_all[:],
                    in_=ag_r[:, :, NT : NT + 2].bitcast(F32).squeeze(),
                )

            def emit_group_coefs(g):
                gs = slice(g * GB, (g + 1) * GB)
                omaK = psm.tile([P, GB], F32, tag="omaK")
                ra = psm.tile([P, GB], F32, tag="ra")
                rzg = psm.tile([P, GB], F32, tag="rzg")
                t1 = psm.tile([P, GB], F32, tag="t1")
                nc.vector.tensor_scalar(
                    out=omaK[:],
                    in0=a_all[:, gs],
                    scalar1=-K,
                    scalar2=K,
                    op0=ALU.mult,
                    op1=ALU.add,
                )
                nc.vector.reciprocal(ra[:], a_all[:, gs])
                nc.vector.reciprocal(rzg[:], zg_sb[:, gs])
                nc.vector.tensor_tensor(out=t1[:], in0=omaK[:], in1=rzg[:], op=ALU.mult)
                nc.vector.tensor_tensor(
                    out=rc2K_all[:, gs], in0=t1[:], in1=ra[:], op=ALU.mult
                )
                nc.vector.tensor_scalar_mul(aK_all[:, gs], a_all[:, gs], 1.0 / K)

            def emit_pass2_batch(b):
                onehot = poh.tile([P, VSP], F16, tag="oh")
                nc.vector.tensor_scalar(
                    out=onehot[:],
                    in0=iota_all[:],
                    scalar1=src_sb[:, b : b + 1],
                    scalar2=K,
                    op0=ALU.is_equal,
                    op1=ALU.mult,
                )
                diag = pdg.tile([P, P], F16, tag="dg")
                nc.scalar.activation(
                    out=diag[:],
                    in_=identity[:],
                    func=AF.Copy,
                    scale=rc2K_all[:, b : b + 1],
                )
                e_b = e_tiles[b]
                for p in range(NPAIR):
                    cp_ps = ps_cp.tile([P, PW], F32, tag="c")
                    for h, col in ((0, slice(0, CH)), (1, slice(CH, PW))):
                        vcol = slice(p * PW + h * CH, p * PW + (h + 1) * CH)
                        nc.tensor.matmul(
                            out=cp_ps[:, col],
                            lhsT=attnT_all[:, b, :],
                            rhs=onehot[:, vcol],
                            start=True,
                            stop=False,
                        )
                        nc.tensor.matmul(
                            out=cp_ps[:, col],
                            lhsT=diag[:],
                            rhs=e_b[:, vcol],
                            start=False,
                            stop=True,
                        )
                    outt = pot.tile([P, PW], F32, tag="ot")
                    nc.scalar.activation(
                        out=outt[:],
                        in_=cp_ps[:],
                        func=AF.Ln,
                        scale=aK_all[:, b : b + 1],
                    )
                    w = min(VS - p * PW, PW)
                    nc.sync.dma_start(
                        out=out[:, b, p * PW : p * PW + w], in_=outt[:, 0:w]
                    )

            # attention + allgather first; then all of pass 1 (PE never
            # head-of-line blocks on the collectives), then all of pass 2
            emit_attention_own()
            for b in range(GB):
                emit_pass1_batch(b)
            emit_group_z(0)
            emit_unpack()
            for b in range(GB, BS):
                emit_pass1_batch(b)
            emit_group_z(1)
            emit_group_coefs(0)
            for b in range(GB):
                emit_pass2_batch(b)
            emit_group_coefs(1)
            for b in range(GB, BS):
                emit_pass2_batch(b)


_NC_CACHE = []


def _get_nc():
    if not _NC_CACHE:
        _NC_CACHE.append(build_kernel())
    return _NC_CACHE[0]


def _make_in_maps(inputs):
    htgt = np.asarray(inputs["htgt"], dtype=np.float32)
    hsrc = np.asarray(inputs["hsrc"], dtype=np.float32)
    src = np.asarray(inputs["src"]).astype(np.int64)
    emb = np.asarray(inputs["emb_weight"], dtype=np.float32)
    q_w = np.asarray(inputs["q_w"], dtype=np.float32)
    q_b = np.asarray(inputs["q_b"], dtype=np.float32)
    f_w = np.asarray(inputs["f_w"], dtype=np.float32)
    f_b = np.asarray(inputs["f_b"], dtype=np.float32)
    copy_w = np.asarray(inputs["copy_w"], dtype=np.float32)
    copy_b = np.asarray(inputs["copy_b"], dtype=np.float32)

    # shared across cores
    htgtT_h = np.ascontiguousarray(
        htgt.transpose(2, 1, 0).astype(np.float16).reshape(KC, P, BS, P)
    )
    qwT_h = np.ascontiguousarray(q_w.T).astype(np.float16).reshape(KC, P, D)
    qb_h = np.ascontiguousarray(q_b.astype(np.float16).reshape(1, D))
    # fold f_w/copy_w:  w2 = f_w.T @ copy_w.T ;  b2 = copy_w @ f_b + copy_b
    w2_full = f_w.T @ copy_w[0]  # [D]
    w2_h = np.ascontiguousarray(w2_full.reshape(KC, P).T.astype(np.float16))
    nb2 = -(copy_w[0] @ f_b + copy_b[0])
    nb2_h = np.ascontiguousarray(
        np.full((P, 1), nb2, dtype=np.float32)
    )

    in_maps = []
    for c in range(NCORES):
        eT = np.zeros((D, VSP), dtype=np.float16)
        eT[:, 0:VS] = emb[c * VS : (c + 1) * VS].T.astype(np.float16)
        embT_h = np.ascontiguousarray(eT.reshape(KC, P, VSP))
        hh_h = np.ascontiguousarray(
            np.stack([htgt[:, c, :].T, hsrc[:, c, :].T], axis=1)
            .astype(np.float16)
            .reshape(KC, P, 2, P)
        )
        # integral values, exact in fp32 (scalar operand of is_equal is f32)
        src_local = np.ascontiguousarray((src - c * VS).astype(np.float32))
        in_maps.append(
            {
                "embT_h": embT_h,
                "htgtT_h": htgtT_h,
                "hh_h": hh_h,
                "qwT_h": qwT_h,
                "qb_h": qb_h,
                "src_h": src_local,
                "w2_h": w2_h,
                "nb2_h": nb2_h,
            }
        )
    return in_maps


def kernel(**inputs):
    in_maps = _make_in_maps(inputs)
    nc = _get_nc()
    res = run_bass_kernel_spmd(nc, in_maps, list(range(NCORES))).results
    return np.concatenate([res[c]["out"] for c in range(NCORES)], axis=2)
